# Initial kernel scaffold
#
"""Trainium2 Bass kernel for nn_MixedFFN (dense FFN + per-token grouped FFN).

Sharding (8 cores, no collectives):
  - seq path: data-parallel over batch. Core i computes the full dense FFN
    for sequence_tokens[i] (4096 tokens, 1024 -> 4096 -> 1024).
  - ns path: expert-parallel. Core i computes experts {2i, 2i+1} of the
    grouped FFN for all 8 batch rows (weights for an expert are read once
    per job instead of 8 times).

Device math is bf16 matmul with fp32 PSUM accumulation; outputs fp32.
Host-side prep only reshapes / casts / transposes (layout, no FLOPs).
"""

import sys
from contextlib import ExitStack

import numpy as np
import ml_dtypes

if "/opt/trn_rl_repo" not in sys.path:
    sys.path.insert(0, "/opt/trn_rl_repo")

BF16 = ml_dtypes.bfloat16

# Problem dims (full inputs)
B = 8
S = 4096
HIDDEN = 1024
FFN = 4096
NS = 16
NCORES = 8
EPC = NS // NCORES  # experts per core

# Controlled by test.py for profiling; harness just calls kernel().
TRACE = False
LAST_RESULT = None


def _dims_full():
    return dict(hidden=HIDDEN, ffn=FFN, s=S, b=B, epc=EPC)


def _emit(nc, tile, mybir, dims):
    """Emit the per-core Tile program. Returns nothing; tensors are declared here."""
    f32 = mybir.dt.float32
    bf16 = mybir.dt.bfloat16
    ACT = mybir.ActivationFunctionType

    hidden, ffn, s, b, epc = (
        dims["hidden"], dims["ffn"], dims["s"], dims["b"], dims["epc"]
    )
    KC = hidden // 128          # contraction chunks for up-proj
    NF = ffn // 128             # ffn blocks of 128 (up-proj output partitions)
    FC = ffn // 128             # contraction chunks for down-proj
    TOK = min(512, s)           # token tile (moving free dim of up-proj)
    NTT = s // TOK              # token tiles
    NMT = TOK // 128            # 128-token subtiles per token tile
    HH = min(512, hidden)       # hidden slab (moving free dim of down-proj)
    NHH = hidden // HH
    FO = min(512, ffn)          # ffn slab for ns up-proj moving dim
    NFO = ffn // FO

    # ---- DRAM tensors -------------------------------------------------
    xT = nc.dram_tensor("xT", (hidden, s), bf16, kind="ExternalInput").ap()
    wup = nc.dram_tensor("wup", (hidden, ffn), bf16, kind="ExternalInput").ap()
    wdn = nc.dram_tensor("wdn", (ffn, hidden), bf16, kind="ExternalInput").ap()
    bup = nc.dram_tensor("bup", (128, NF), f32, kind="ExternalInput").ap()
    bdn = nc.dram_tensor("bdn", (128, hidden), f32, kind="ExternalInput").ap()
    msk = nc.dram_tensor("msk", (128, s // 128), f32, kind="ExternalInput").ap()
    xn = nc.dram_tensor("xn", (epc, hidden, b), bf16, kind="ExternalInput").ap()
    wu_n = nc.dram_tensor("wu_n", (epc, hidden, ffn), bf16, kind="ExternalInput").ap()
    wd_n = nc.dram_tensor("wd_n", (epc, ffn, hidden), bf16, kind="ExternalInput").ap()
    bu_n = nc.dram_tensor("bu_n", (epc, b, ffn), bf16, kind="ExternalInput").ap()
    bd_n = nc.dram_tensor("bd_n", (epc, b, hidden), f32, kind="ExternalInput").ap()
    nm = nc.dram_tensor("nm", (epc, b, 1), f32, kind="ExternalInput").ap()
    idb = nc.dram_tensor("idb", (b, b), bf16, kind="ExternalInput").ap()
    yseq = nc.dram_tensor("yseq", (s, hidden), f32, kind="ExternalOutput").ap()
    yns = nc.dram_tensor("yns", (epc, b, hidden), f32, kind="ExternalOutput").ap()

    with tile.TileContext(nc) as tc, ExitStack() as ctx:
        const_pool = ctx.enter_context(tc.tile_pool(name="const", bufs=1))
        xt_pool = ctx.enter_context(tc.tile_pool(name="xt", bufs=2))
        ht_pool = ctx.enter_context(tc.tile_pool(name="ht", bufs=1))
        y_pool = ctx.enter_context(tc.tile_pool(name="y", bufs=3))

        bup_sb = const_pool.tile([128, NF], f32)
        nc.sync.dma_start(bup_sb[:], bup[:])
        bdn_sb = const_pool.tile([128, hidden], f32)
        nc.sync.dma_start(bdn_sb[:], bdn[:])
        msk_sb = const_pool.tile([128, s // 128], f32)
        nc.sync.dma_start(msk_sb[:], msk[:])
        id_sb = const_pool.tile([b, b], bf16)
        nc.sync.dma_start(id_sb[:], idb[:])

        # ================= dense sequence FFN =================
        with (
            tc.tile_pool(name="wseq", bufs=1) as wpool,
            tc.tile_pool(name="ph", bufs=3, space="PSUM") as ph_pool,
            tc.tile_pool(name="py", bufs=4, space="PSUM") as py_pool,
        ):
            wup_sb = wpool.tile([128, KC * ffn], bf16)
            for k in range(KC):
                nc.sync.dma_start(
                    wup_sb[:, k * ffn:(k + 1) * ffn], wup[k * 128:(k + 1) * 128, :]
                )
            wdn_sb = wpool.tile([128, FC * hidden], bf16)
            for c in range(FC):
                nc.sync.dma_start(
                    wdn_sb[:, c * hidden:(c + 1) * hidden],
                    wdn[c * 128:(c + 1) * 128, :],
                )

            for tt in range(NTT):
                xt_sb = xt_pool.tile([128, KC * TOK], bf16)
                for k in range(KC):
                    nc.sync.dma_start(
                        xt_sb[:, k * TOK:(k + 1) * TOK],
                        xT[k * 128:(k + 1) * 128, tt * TOK:(tt + 1) * TOK],
                    )
                ht_sb = ht_pool.tile([128, NF * TOK], bf16)
                # up-proj: psum[ffn_block 128, TOK] += wup_chunk.T @ xT_chunk
                for f in range(NF):
                    psh = ph_pool.tile([128, TOK], f32)
                    for k in range(KC):
                        nc.tensor.matmul(
                            psh[:],
                            wup_sb[:, k * ffn + f * 128: k * ffn + f * 128 + 128],
                            xt_sb[:, k * TOK:(k + 1) * TOK],
                            start=(k == 0),
                            stop=(k == KC - 1),
                        )
                    # silu(h + b_up) with per-partition bias, cast to bf16
                    nc.scalar.activation(
                        ht_sb[:, f * TOK:(f + 1) * TOK],
                        psh[:],
                        ACT.Silu,
                        bias=bup_sb[:, f:f + 1],
                    )
                # down-proj: psum[tok 128, HH] += hT_chunk.T @ wdn_chunk
                for mt in range(NMT):
                    for hh in range(NHH):
                        psy = py_pool.tile([128, HH], f32)
                        for c in range(FC):
                            nc.tensor.matmul(
                                psy[:],
                                ht_sb[:, c * TOK + mt * 128: c * TOK + mt * 128 + 128],
                                wdn_sb[:, c * hidden + hh * HH: c * hidden + (hh + 1) * HH],
                                start=(c == 0),
                                stop=(c == FC - 1),
                            )
                        y_sb = y_pool.tile([128, HH], f32)
                        nc.vector.tensor_add(
                            y_sb[:], psy[:], bdn_sb[:, hh * HH:(hh + 1) * HH]
                        )
                        jblk = tt * NMT + mt
                        nc.vector.tensor_scalar_mul(
                            y_sb[:], y_sb[:], msk_sb[:, jblk:jblk + 1]
                        )
                        nc.sync.dma_start(
                            yseq[tt * TOK + mt * 128: tt * TOK + (mt + 1) * 128,
                                 hh * HH:(hh + 1) * HH],
                            y_sb[:],
                        )

        # ================= grouped (ns) FFN =================
        with (
            tc.tile_pool(name="nsw", bufs=6) as nsw_pool,
            tc.tile_pool(name="nsh", bufs=1) as nsh_pool,
            tc.tile_pool(name="nsc", bufs=2) as nsc_pool,
            tc.tile_pool(name="nst", bufs=2) as nst_pool,
            tc.tile_pool(name="nps", bufs=2, space="PSUM") as nps_pool,
            tc.tile_pool(name="tps", bufs=2, space="PSUM") as tps_pool,
        ):
            for e in range(epc):
                xne_sb = nsc_pool.tile([128, KC * b], bf16)
                for k in range(KC):
                    nc.sync.dma_start(
                        xne_sb[:, k * b:(k + 1) * b],
                        xn[e, k * 128:(k + 1) * 128, :],
                    )
                bue_sb = nsc_pool.tile([b, ffn], bf16)
                nc.sync.dma_start(bue_sb[:], bu_n[e])
                h_sb = nsh_pool.tile([b, ffn], bf16)
                # up-proj: psum[b, FO] += xne_chunk.T @ wu_chunk  (tokens stationary)
                for fo in range(NFO):
                    psn = nps_pool.tile([b, FO], f32)
                    for k in range(KC):
                        wt = nsw_pool.tile([128, FO], bf16, tag="wt")
                        nc.sync.dma_start(
                            wt[:],
                            wu_n[e, k * 128:(k + 1) * 128, fo * FO:(fo + 1) * FO],
                        )
                        nc.tensor.matmul(
                            psn[:],
                            xne_sb[:, k * b:(k + 1) * b],
                            wt[:],
                            start=(k == 0),
                            stop=(k == KC - 1),
                        )
                    tmpn = nst_pool.tile([b, FO], f32)
                    nc.vector.tensor_add(
                        tmpn[:], psn[:], bue_sb[:, fo * FO:(fo + 1) * FO]
                    )
                    nc.scalar.activation(
                        h_sb[:, fo * FO:(fo + 1) * FO], tmpn[:], ACT.Silu
                    )
                # transpose h [b, ffn] -> hT [ffn, b] in 128-col chunks (PE transpose)
                hT_sb = nsc_pool.tile([128, FC * b], bf16)
                for c in range(FC):
                    pt = tps_pool.tile([128, b], bf16)
                    nc.tensor.transpose(
                        pt[:], h_sb[:, c * 128:(c + 1) * 128], id_sb[:]
                    )
                    nc.vector.tensor_copy(hT_sb[:, c * b:(c + 1) * b], pt[:])
                # down-proj: psum[b, HH] += hT_chunk.T @ wd_chunk
                bde_sb = nsc_pool.tile([b, hidden], f32)
                nc.sync.dma_start(bde_sb[:], bd_n[e])
                nme_sb = nsc_pool.tile([b, 1], f32)
                nc.sync.dma_start(nme_sb[:], nm[e])
                for hh in range(NHH):
                    psn2 = nps_pool.tile([b, HH], f32, tag="psn2")
                    for c in range(FC):
                        wt = nsw_pool.tile([128, HH], bf16, tag="wt")
                        nc.sync.dma_start(
                            wt[:],
                            wd_n[e, c * 128:(c + 1) * 128, hh * HH:(hh + 1) * HH],
                        )
                        nc.tensor.matmul(
                            psn2[:],
                            hT_sb[:, c * b:(c + 1) * b],
                            wt[:],
                            start=(c == 0),
                            stop=(c == FC - 1),
                        )
                    yn_sb = nst_pool.tile([b, HH], f32, tag="yn")
                    nc.vector.tensor_add(
                        yn_sb[:], psn2[:], bde_sb[:, hh * HH:(hh + 1) * HH]
                    )
                    nc.vector.tensor_scalar_mul(yn_sb[:], yn_sb[:], nme_sb[:])
                    nc.sync.dma_start(yns[e, :, hh * HH:(hh + 1) * HH], yn_sb[:])


_NC_CACHE = {}


def _get_nc(dims_key="full"):
    if dims_key not in _NC_CACHE:
        import concourse.bass as bass
        import concourse.tile as tile
        import concourse.mybir as mybir

        nc = bass.Bass("TRN2", debug=False)
        _emit(nc, tile, mybir, _dims_full())
        _NC_CACHE[dims_key] = nc
    return _NC_CACHE[dims_key]


def _shard_inputs(inputs):
    """Host-side layout prep: slice/cast/transpose. Returns list of 8 in_maps."""
    f32 = np.float32
    seq_tok = np.asarray(inputs["sequence_tokens"], f32)
    seq_mask = np.asarray(inputs["sequence_mask"])
    ns_tok = np.asarray(inputs["ns_tokens"], f32)
    ns_mask = np.asarray(inputs["ns_mask"])
    wup = np.asarray(inputs["seq_up_w"], f32).astype(BF16)
    bup_v = np.asarray(inputs["seq_up_b"], f32)
    wdn = np.asarray(inputs["seq_down_w"], f32).astype(BF16)
    bdn_v = np.asarray(inputs["seq_down_b"], f32)
    wu_ns = np.asarray(inputs["ns_up_weight"], f32)
    bu_ns = np.asarray(inputs["ns_up_bias"], f32)
    wd_ns = np.asarray(inputs["ns_down_weight"], f32)
    bd_ns = np.asarray(inputs["ns_down_bias"], f32)

    bup_h = np.ascontiguousarray(bup_v.reshape(FFN // 128, 128).T)
    bdn_h = np.ascontiguousarray(np.broadcast_to(bdn_v, (128, HIDDEN)))
    idb = np.eye(B, dtype=BF16)

    in_maps = []
    for i in range(NCORES):
        es = slice(i * EPC, (i + 1) * EPC)
        m = {
            "xT": seq_tok[i].T.astype(BF16),
            "wup": wup,
            "wdn": wdn,
            "bup": bup_h,
            "bdn": bdn_h,
            "msk": np.ascontiguousarray(
                seq_mask[i].astype(f32).reshape(S // 128, 128).T
            ),
            "xn": np.ascontiguousarray(ns_tok[:, es, :].transpose(1, 2, 0)).astype(BF16),
            "wu_n": wu_ns[es].astype(BF16),
            "wd_n": wd_ns[es].astype(BF16),
            "bu_n": np.ascontiguousarray(
                np.broadcast_to(bu_ns[es][:, None, :], (EPC, B, FFN))
            ).astype(BF16),
            "bd_n": np.ascontiguousarray(
                np.broadcast_to(bd_ns[es][:, None, :], (EPC, B, HIDDEN))
            ),
            "nm": np.ascontiguousarray(
                ns_mask[:, es].astype(f32).T.reshape(EPC, B, 1)
            ),
            "idb": idb,
        }
        in_maps.append(m)
    return in_maps


def kernel(**inputs):
    global LAST_RESULT
    from concourse.bass_utils import run_bass_kernel_spmd

    nc = _get_nc()
    in_maps = _shard_inputs(inputs)
    res = run_bass_kernel_spmd(
        nc, in_maps, core_ids=list(range(NCORES)), trace=TRACE
    )
    LAST_RESULT = res

    seq_out = np.stack(
        [np.asarray(res.results[i]["yseq"], np.float32) for i in range(NCORES)], axis=0
    )
    ns_out = np.empty((B, NS, HIDDEN), np.float32)
    for i in range(NCORES):
        r = np.asarray(res.results[i]["yns"], np.float32)  # (EPC, B, HIDDEN)
        for e in range(EPC):
            ns_out[:, i * EPC + e, :] = r[e]
    return seq_out, ns_out


# revision 9
# speedup vs baseline: 1.5010x; 1.5010x over previous
"""Trainium2 Bass kernel for nn_MixedFFN (dense FFN + per-token grouped FFN).

Sharding (8 cores, no collectives):
  - seq path: data-parallel over batch. Core i computes the full dense FFN
    for sequence_tokens[i] (4096 tokens, 1024 -> 4096 -> 1024).
  - ns path: expert-parallel. Core i computes experts {2i, 2i+1} of the
    grouped FFN for all 8 batch rows (weights for an expert are read once
    per job instead of 8 times).

Device math is bf16 matmul with fp32 PSUM accumulation; outputs fp32.
Host-side prep only reshapes / casts / transposes (layout, no FLOPs).
"""

import sys
from contextlib import ExitStack

import numpy as np
import ml_dtypes

if "/opt/trn_rl_repo" not in sys.path:
    sys.path.insert(0, "/opt/trn_rl_repo")

BF16 = ml_dtypes.bfloat16

# Problem dims (full inputs)
B = 8
S = 4096
HIDDEN = 1024
FFN = 4096
NS = 16
NCORES = 8
EPC = NS // NCORES  # experts per core

LAST_RESULT = None


def _dims_full():
    return dict(hidden=HIDDEN, ffn=FFN, s=S, b=B, epc=EPC)


def _emit(nc, tile, mybir, dims):
    """Emit the per-core Tile program. Returns nothing; tensors are declared here."""
    f32 = mybir.dt.float32
    bf16 = mybir.dt.bfloat16
    ACT = mybir.ActivationFunctionType

    hidden, ffn, s, b, epc = (
        dims["hidden"], dims["ffn"], dims["s"], dims["b"], dims["epc"]
    )
    KC = hidden // 128          # contraction chunks for up-proj
    NF = ffn // 128             # ffn blocks of 128 (up-proj output partitions)
    FC = ffn // 128             # contraction chunks for down-proj
    TOK = min(512, s)           # token tile (moving free dim of up-proj)
    NTT = s // TOK              # token tiles
    NMT = TOK // 128            # 128-token subtiles per token tile
    HH = min(512, hidden)       # hidden slab (moving free dim of down-proj)
    NHH = hidden // HH
    FO = min(512, ffn)          # ffn slab for ns up-proj moving dim
    NFO = ffn // FO

    # ---- DRAM tensors -------------------------------------------------
    xT = nc.dram_tensor("xT", (hidden, s), bf16, kind="ExternalInput").ap()
    wup = nc.dram_tensor("wup", (hidden, ffn), bf16, kind="ExternalInput").ap()
    wdn = nc.dram_tensor("wdn", (ffn, hidden), bf16, kind="ExternalInput").ap()
    bup = nc.dram_tensor("bup", (128, NF), f32, kind="ExternalInput").ap()
    bdn = nc.dram_tensor("bdn", (128, hidden), f32, kind="ExternalInput").ap()
    msk = nc.dram_tensor("msk", (128, s // 128), f32, kind="ExternalInput").ap()
    xn = nc.dram_tensor("xn", (epc, hidden, b), bf16, kind="ExternalInput").ap()
    wu_n = nc.dram_tensor("wu_n", (epc, hidden, ffn), bf16, kind="ExternalInput").ap()
    wd_n = nc.dram_tensor("wd_n", (epc, ffn, hidden), bf16, kind="ExternalInput").ap()
    bu_n = nc.dram_tensor("bu_n", (epc, b, ffn), bf16, kind="ExternalInput").ap()
    bd_n = nc.dram_tensor("bd_n", (epc, b, hidden), f32, kind="ExternalInput").ap()
    nm = nc.dram_tensor("nm", (epc, b, 1), f32, kind="ExternalInput").ap()
    idb = nc.dram_tensor("idb", (b, b), bf16, kind="ExternalInput").ap()
    yseq = nc.dram_tensor("yseq", (s, hidden), f32, kind="ExternalOutput").ap()
    yns = nc.dram_tensor("yns", (epc, b, hidden), f32, kind="ExternalOutput").ap()

    with tile.TileContext(nc) as tc, ExitStack() as ctx:
        const_pool = ctx.enter_context(tc.tile_pool(name="const", bufs=1))
        xt_pool = ctx.enter_context(tc.tile_pool(name="xt", bufs=2))
        ht_pool = ctx.enter_context(tc.tile_pool(name="ht", bufs=1))
        y_pool = ctx.enter_context(tc.tile_pool(name="y", bufs=3))

        bup_sb = const_pool.tile([128, NF], f32)
        nc.sync.dma_start(bup_sb[:], bup[:])
        bdn_sb = const_pool.tile([128, hidden], f32)
        nc.sync.dma_start(bdn_sb[:], bdn[:])
        msk_sb = const_pool.tile([128, s // 128], f32)
        nc.sync.dma_start(msk_sb[:], msk[:])
        id_sb = const_pool.tile([b, b], bf16)
        nc.sync.dma_start(id_sb[:], idb[:])

        # ================= dense sequence FFN =================
        with (
            tc.tile_pool(name="wseq", bufs=1) as wpool,
            tc.tile_pool(name="ph", bufs=3, space="PSUM") as ph_pool,
            tc.tile_pool(name="py", bufs=4, space="PSUM") as py_pool,
        ):
            wup_sb = wpool.tile([128, KC * ffn], bf16)
            for k in range(KC):
                nc.sync.dma_start(
                    wup_sb[:, k * ffn:(k + 1) * ffn], wup[k * 128:(k + 1) * 128, :]
                )
            wdn_sb = wpool.tile([128, FC * hidden], bf16)
            for c in range(FC):
                nc.sync.dma_start(
                    wdn_sb[:, c * hidden:(c + 1) * hidden],
                    wdn[c * 128:(c + 1) * 128, :],
                )

            for tt in range(NTT):
                xt_sb = xt_pool.tile([128, KC * TOK], bf16)
                for k in range(KC):
                    nc.sync.dma_start(
                        xt_sb[:, k * TOK:(k + 1) * TOK],
                        xT[k * 128:(k + 1) * 128, tt * TOK:(tt + 1) * TOK],
                    )
                ht_sb = ht_pool.tile([128, NF * TOK], bf16)
                # up-proj: psum[ffn_block 128, TOK] += wup_chunk.T @ xT_chunk
                for f in range(NF):
                    psh = ph_pool.tile([128, TOK], f32)
                    for k in range(KC):
                        nc.tensor.matmul(
                            psh[:],
                            wup_sb[:, k * ffn + f * 128: k * ffn + f * 128 + 128],
                            xt_sb[:, k * TOK:(k + 1) * TOK],
                            start=(k == 0),
                            stop=(k == KC - 1),
                        )
                    # silu(h + b_up) with per-partition bias, cast to bf16
                    nc.scalar.activation(
                        ht_sb[:, f * TOK:(f + 1) * TOK],
                        psh[:],
                        ACT.Silu,
                        bias=bup_sb[:, f:f + 1],
                    )
                # down-proj: psum[tok 128, HH] += hT_chunk.T @ wdn_chunk
                for mt in range(NMT):
                    for hh in range(NHH):
                        psy = py_pool.tile([128, HH], f32)
                        for c in range(FC):
                            nc.tensor.matmul(
                                psy[:],
                                ht_sb[:, c * TOK + mt * 128: c * TOK + mt * 128 + 128],
                                wdn_sb[:, c * hidden + hh * HH: c * hidden + (hh + 1) * HH],
                                start=(c == 0),
                                stop=(c == FC - 1),
                            )
                        y_sb = y_pool.tile([128, HH], f32)
                        nc.vector.tensor_add(
                            y_sb[:], psy[:], bdn_sb[:, hh * HH:(hh + 1) * HH]
                        )
                        jblk = tt * NMT + mt
                        nc.vector.tensor_scalar_mul(
                            y_sb[:], y_sb[:], msk_sb[:, jblk:jblk + 1]
                        )
                        nc.sync.dma_start(
                            yseq[tt * TOK + mt * 128: tt * TOK + (mt + 1) * 128,
                                 hh * HH:(hh + 1) * HH],
                            y_sb[:],
                        )

        # ================= grouped (ns) FFN =================
        with (
            tc.tile_pool(name="nsw", bufs=6) as nsw_pool,
            tc.tile_pool(name="nsh", bufs=1) as nsh_pool,
            tc.tile_pool(name="nsc", bufs=2) as nsc_pool,
            tc.tile_pool(name="nst", bufs=2) as nst_pool,
            tc.tile_pool(name="nps", bufs=2, space="PSUM") as nps_pool,
            tc.tile_pool(name="tps", bufs=2, space="PSUM") as tps_pool,
        ):
            for e in range(epc):
                xne_sb = nsc_pool.tile([128, KC * b], bf16)
                for k in range(KC):
                    nc.sync.dma_start(
                        xne_sb[:, k * b:(k + 1) * b],
                        xn[e, k * 128:(k + 1) * 128, :],
                    )
                bue_sb = nsc_pool.tile([b, ffn], bf16)
                nc.sync.dma_start(bue_sb[:], bu_n[e])
                h_sb = nsh_pool.tile([b, ffn], bf16)
                # up-proj: psum[b, FO] += xne_chunk.T @ wu_chunk  (tokens stationary)
                for fo in range(NFO):
                    psn = nps_pool.tile([b, FO], f32)
                    for k in range(KC):
                        wt = nsw_pool.tile([128, FO], bf16, tag="wt")
                        nc.sync.dma_start(
                            wt[:],
                            wu_n[e, k * 128:(k + 1) * 128, fo * FO:(fo + 1) * FO],
                        )
                        nc.tensor.matmul(
                            psn[:],
                            xne_sb[:, k * b:(k + 1) * b],
                            wt[:],
                            start=(k == 0),
                            stop=(k == KC - 1),
                        )
                    tmpn = nst_pool.tile([b, FO], f32)
                    nc.vector.tensor_add(
                        tmpn[:], psn[:], bue_sb[:, fo * FO:(fo + 1) * FO]
                    )
                    nc.scalar.activation(
                        h_sb[:, fo * FO:(fo + 1) * FO], tmpn[:], ACT.Silu
                    )
                # transpose h [b, ffn] -> hT [ffn, b] in 128-col chunks (PE transpose)
                hT_sb = nsc_pool.tile([128, FC * b], bf16)
                for c in range(FC):
                    pt = tps_pool.tile([128, b], bf16)
                    nc.tensor.transpose(
                        pt[:], h_sb[:, c * 128:(c + 1) * 128], id_sb[:]
                    )
                    nc.vector.tensor_copy(hT_sb[:, c * b:(c + 1) * b], pt[:])
                # down-proj: psum[b, HH] += hT_chunk.T @ wd_chunk
                bde_sb = nsc_pool.tile([b, hidden], f32)
                nc.sync.dma_start(bde_sb[:], bd_n[e])
                nme_sb = nsc_pool.tile([b, 1], f32)
                nc.sync.dma_start(nme_sb[:], nm[e])
                for hh in range(NHH):
                    psn2 = nps_pool.tile([b, HH], f32, tag="psn2")
                    for c in range(FC):
                        wt = nsw_pool.tile([128, HH], bf16, tag="wt")
                        nc.sync.dma_start(
                            wt[:],
                            wd_n[e, c * 128:(c + 1) * 128, hh * HH:(hh + 1) * HH],
                        )
                        nc.tensor.matmul(
                            psn2[:],
                            hT_sb[:, c * b:(c + 1) * b],
                            wt[:],
                            start=(c == 0),
                            stop=(c == FC - 1),
                        )
                    yn_sb = nst_pool.tile([b, HH], f32, tag="yn")
                    nc.vector.tensor_add(
                        yn_sb[:], psn2[:], bde_sb[:, hh * HH:(hh + 1) * HH]
                    )
                    nc.vector.tensor_scalar_mul(yn_sb[:], yn_sb[:], nme_sb[:])
                    nc.sync.dma_start(yns[e, :, hh * HH:(hh + 1) * HH], yn_sb[:])


_NC_CACHE = {}


def _get_nc(dims_key="full"):
    if dims_key not in _NC_CACHE:
        import concourse.bacc as bacc
        import concourse.tile as tile
        import concourse.mybir as mybir

        nc = bacc.Bacc("TRN2", debug=False)
        _emit(nc, tile, mybir, _dims_full())
        nc.finalize()
        _NC_CACHE[dims_key] = nc
    return _NC_CACHE[dims_key]


def _shard_inputs(inputs):
    """Host-side layout prep: slice/cast/transpose. Returns list of 8 in_maps."""
    f32 = np.float32
    seq_tok = np.asarray(inputs["sequence_tokens"], f32)
    seq_mask = np.asarray(inputs["sequence_mask"])
    ns_tok = np.asarray(inputs["ns_tokens"], f32)
    ns_mask = np.asarray(inputs["ns_mask"])
    wup = np.asarray(inputs["seq_up_w"], f32).astype(BF16)
    bup_v = np.asarray(inputs["seq_up_b"], f32)
    wdn = np.asarray(inputs["seq_down_w"], f32).astype(BF16)
    bdn_v = np.asarray(inputs["seq_down_b"], f32)
    wu_ns = np.asarray(inputs["ns_up_weight"], f32)
    bu_ns = np.asarray(inputs["ns_up_bias"], f32)
    wd_ns = np.asarray(inputs["ns_down_weight"], f32)
    bd_ns = np.asarray(inputs["ns_down_bias"], f32)

    bup_h = np.ascontiguousarray(bup_v.reshape(FFN // 128, 128).T)
    bdn_h = np.ascontiguousarray(np.broadcast_to(bdn_v, (128, HIDDEN)))
    idb = np.eye(B, dtype=BF16)

    in_maps = []
    for i in range(NCORES):
        es = slice(i * EPC, (i + 1) * EPC)
        m = {
            "xT": seq_tok[i].T.astype(BF16),
            "wup": wup,
            "wdn": wdn,
            "bup": bup_h,
            "bdn": bdn_h,
            "msk": np.ascontiguousarray(
                seq_mask[i].astype(f32).reshape(S // 128, 128).T
            ),
            "xn": np.ascontiguousarray(ns_tok[:, es, :].transpose(1, 2, 0)).astype(BF16),
            "wu_n": wu_ns[es].astype(BF16),
            "wd_n": wd_ns[es].astype(BF16),
            "bu_n": np.ascontiguousarray(
                np.broadcast_to(bu_ns[es][:, None, :], (EPC, B, FFN))
            ).astype(BF16),
            "bd_n": np.ascontiguousarray(
                np.broadcast_to(bd_ns[es][:, None, :], (EPC, B, HIDDEN))
            ),
            "nm": np.ascontiguousarray(
                ns_mask[:, es].astype(f32).T.reshape(EPC, B, 1)
            ),
            "idb": idb,
        }
        in_maps.append(m)
    return in_maps


class _Runner:
    """Cached jitted SPMD runner over 8 cores (replicates
    bass2jax.run_bass_via_pjrt's multi-core path so the jit compile is
    done once per process and repeated calls only execute)."""

    def __init__(self, nc):
        import jax
        import concourse.mybir as mybir
        from concourse import bass2jax
        from jax.experimental.shard_map import shard_map
        from jax.sharding import Mesh, PartitionSpec

        bass2jax.install_neuronx_cc_hook()
        self.jax = jax
        self.nc = nc

        in_names, out_names, out_avals = [], [], []
        partition_name = (
            nc.partition_id_tensor.name if nc.partition_id_tensor else None
        )
        for alloc in nc.m.functions[0].allocations:
            if not isinstance(alloc, mybir.MemoryLocationSet):
                continue
            name = alloc.memorylocations[0].name
            if alloc.kind == "ExternalInput":
                if name != partition_name:
                    in_names.append(name)
            elif alloc.kind == "ExternalOutput":
                out_names.append(name)
                out_avals.append(
                    jax.core.ShapedArray(
                        tuple(alloc.tensor_shape), mybir.dt.np(alloc.dtype)
                    )
                )
        self.in_names = list(in_names)
        self.out_names = out_names
        self.out_avals = out_avals
        n_params = len(in_names)
        n_outs = len(out_names)
        all_names = in_names + out_names
        if partition_name is not None:
            all_names.append(partition_name)

        def _body(*args):
            operands = list(args)
            if partition_name is not None:
                operands.append(bass2jax.partition_id_tensor())
            outs = bass2jax._bass_exec_p.bind(
                *operands,
                out_avals=tuple(out_avals),
                in_names=tuple(all_names),
                out_names=tuple(out_names),
                lowering_input_output_aliases=(),
                sim_require_finite=True,
                sim_require_nnan=True,
                nc=nc,
            )
            return tuple(outs)

        devices = jax.devices()[:NCORES]
        self.mesh = Mesh(np.asarray(devices), ("core",))
        self.pspec = PartitionSpec("core")
        in_specs = (self.pspec,) * (n_params + n_outs)
        out_specs = (self.pspec,) * n_outs
        self.donate = tuple(range(n_params, n_params + n_outs))
        self.sharded = jax.jit(
            shard_map(
                _body,
                mesh=self.mesh,
                in_specs=in_specs,
                out_specs=out_specs,
                check_rep=False,
            ),
            donate_argnums=self.donate,
            keep_unused=True,
        )
        # Non-donating variant for benchmarking: zero buffers stay alive so
        # they can be reused across calls (outputs are fully written by the
        # NEFF, so initial result-buffer contents don't matter).
        self.sharded_nodon = jax.jit(
            shard_map(
                _body,
                mesh=self.mesh,
                in_specs=in_specs,
                out_specs=out_specs,
                check_rep=False,
            ),
            keep_unused=True,
        )
        from jax.sharding import NamedSharding

        self.sharding = NamedSharding(self.mesh, self.pspec)

        def _zeros():
            return tuple(
                self.jax.numpy.zeros((NCORES * a.shape[0], *a.shape[1:]), a.dtype)
                for a in out_avals
            )

        self.make_zeros = jax.jit(
            _zeros, out_shardings=(self.sharding,) * n_outs
        )

    def concat_inputs(self, in_maps):
        return [
            np.concatenate([in_maps[c][n] for c in range(NCORES)], axis=0)
            for n in self.in_names
        ]

    def put_inputs(self, concat_in):
        return [self.jax.device_put(x, self.sharding) for x in concat_in]

    def run(self, in_dev):
        out_arrs = self.sharded(*in_dev, *self.make_zeros())
        return out_arrs

    def results(self, out_arrs):
        per_core = []
        for c in range(NCORES):
            per_core.append(
                {
                    n: np.asarray(out_arrs[i]).reshape(
                        NCORES, *self.out_avals[i].shape
                    )[c]
                    for i, n in enumerate(self.out_names)
                }
            )
        return per_core


def _make_chain_fn(runner, n):
    """Jitted fn running the NEFF n times serially (data-dependent chain)."""
    import jax
    import jax.numpy as jnp
    from jax.experimental.shard_map import shard_map

    nc = runner.nc
    from concourse import bass2jax

    out_avals = runner.out_avals
    n_params = len(runner.in_names)
    n_outs = len(runner.out_names)
    partition_name = nc.partition_id_tensor.name if nc.partition_id_tensor else None
    all_names = runner.in_names + runner.out_names
    if partition_name is not None:
        all_names = all_names + [partition_name]

    def _bind(ins, zbufs):
        operands = list(ins) + list(zbufs)
        if partition_name is not None:
            operands.append(bass2jax.partition_id_tensor())
        return bass2jax._bass_exec_p.bind(
            *operands,
            out_avals=tuple(out_avals),
            in_names=tuple(all_names),
            out_names=tuple(runner.out_names),
            lowering_input_output_aliases=(),
            sim_require_finite=True,
            sim_require_nnan=True,
            nc=nc,
        )

    def _body_n(*args):
        ins = args[:n_params]
        zbufs = args[n_params:]
        outs = _bind(ins, zbufs)
        for _ in range(n - 1):
            s = outs[0].ravel()[0] * 0.0
            zbufs = tuple(
                jnp.full(a.shape, s, a.dtype) for a in out_avals
            )
            outs = _bind(ins, zbufs)
        return tuple(outs)

    in_specs = (runner.pspec,) * (n_params + n_outs)
    out_specs = (runner.pspec,) * n_outs
    return jax.jit(
        shard_map(
            _body_n,
            mesh=runner.mesh,
            in_specs=in_specs,
            out_specs=out_specs,
            check_rep=False,
        ),
        keep_unused=True,
    )


def bench_chain(inputs, ns=(1, 9), reps=5):
    """Estimate per-execution device time via chained executions."""
    import time

    runner = _get_runner()
    in_maps = _shard_inputs(inputs)
    in_dev = runner.put_inputs(runner.concat_inputs(in_maps))
    zeros = runner.make_zeros()
    runner.jax.block_until_ready(zeros)
    out = None
    stats = {}
    for n in ns:
        fn = _make_chain_fn(runner, n)
        out = fn(*in_dev, *zeros)  # compile + warm
        runner.jax.block_until_ready(out)
        times = []
        for _ in range(reps):
            t0 = time.perf_counter()
            out = fn(*in_dev, *zeros)
            runner.jax.block_until_ready(out)
            times.append(time.perf_counter() - t0)
        stats[n] = sorted(int(t * 1e9) for t in times)
    n_lo, n_hi = min(ns), max(ns)
    slope = (stats[n_hi][0] - stats[n_lo][0]) / (n_hi - n_lo)
    return {
        "stats": stats,
        "per_exec_ns": int(slope),
        "results": _gather(runner.results(out)),
    }


_RUNNER = None


def _get_runner():
    global _RUNNER
    if _RUNNER is None:
        _RUNNER = _Runner(_get_nc())
    return _RUNNER


def _gather(results):
    seq_out = np.stack(
        [np.asarray(results[i]["yseq"], np.float32) for i in range(NCORES)], axis=0
    )
    ns_out = np.empty((B, NS, HIDDEN), np.float32)
    for i in range(NCORES):
        r = np.asarray(results[i]["yns"], np.float32)  # (EPC, B, HIDDEN)
        for e in range(EPC):
            ns_out[:, i * EPC + e, :] = r[e]
    return seq_out, ns_out


def kernel(**inputs):
    runner = _get_runner()
    in_maps = _shard_inputs(inputs)
    out_arrs = runner.run(runner.concat_inputs(in_maps))
    return _gather(runner.results(out_arrs))


def _pipelined_slope(runner, in_dev, zeros, n_lo=4, n_hi=16, reps=3):
    """Per-call time via pipelined async dispatch: slope of total time in n."""
    import time

    jx = runner.jax
    out = runner.sharded_nodon(*in_dev, *zeros)
    jx.block_until_ready(out)
    totals = {}
    for n in (n_lo, n_hi):
        best = None
        for _ in range(reps):
            t0 = time.perf_counter()
            outs = [runner.sharded_nodon(*in_dev, *zeros) for _ in range(n)]
            jx.block_until_ready(outs)
            dt = time.perf_counter() - t0
            best = dt if best is None else min(best, dt)
        totals[n] = best
    slope = (totals[n_hi] - totals[n_lo]) / (n_hi - n_lo)
    return int(slope * 1e9), totals, out


def bench(inputs, iters=3):
    """Estimate device execution time via pipelined slope, and return results."""
    runner = _get_runner()
    in_maps = _shard_inputs(inputs)
    in_dev = runner.put_inputs(runner.concat_inputs(in_maps))
    zeros = runner.make_zeros()
    runner.jax.block_until_ready(zeros)
    slope_ns, totals, out = _pipelined_slope(runner, in_dev, zeros, reps=iters)
    return {
        "median_ns": slope_ns,
        "min_ns": slope_ns,
        "all_ns": [slope_ns],
        "totals": totals,
        "results": _gather(runner.results(out)),
    }
